# revision 32
# baseline (speedup 1.0000x reference)
"""Trainium2 Bass kernel for nn_CLUB_816043786555 (CLUB loss).

Full-input contract: kernel(**inputs) takes the complete arrays, shards the
batch dim across 8 NeuronCores, runs a Bass/Tile kernel per core, and
combines tiny per-core partial sums on the host.

Math: with mu = leaky(x@W1m+b1m)@W2m+b2m, logvar = tanh(leaky(x@W1v+b1v)@W2v+b2v),
iv = exp(-logvar), ym_d = mean_i y, y2m_d = mean_i y^2:

  loss = -0.5/N * sum_{i,d} iv*(y^2 - 2*mu*y - y2m + 2*mu*ym)
       = -0.5/N * [ P1 - 2*P2 - sum_d y2m_d*B_d + 2*sum_d ym_d*C_d ]

with per-core partials P1 = sum iv*y^2, P2 = sum iv*mu*y, C_d = sum_i iv*mu,
B_d = sum_i iv, S_d = sum_i y, T_d = sum_i y^2.  All partials are produced
on-device as fp32 accumulations; the host combine is O(128) work.

Host-side prep (dtype/layout staging only): x/y/W cast to fp16 (identical
rounding to the previous on-device cast path, no DRAM bounce), weights packed
into a single [128,2048] fp16 tensor and biases into [128,10] f32 so startup
is 2 DMAs, b2v negated for tanh's bias slot.

Schedule: groups of RG=1024 rows flow through a 3-deep software pipeline.
Unit g emits:
  - transpose-loads for group g+1
  - product stage for group g-2 (all inputs are then a full unit old, so no
    engine ever head-blocks its in-order queue on a cross-engine dependency)
  - L1(g) matmuls interleaved per 128-wide hidden chunk with L2(g-1)
    matmuls so the PE never gaps (its p-state ramp resets on any gap);
    L2's z-head occupies interleave slots 0-1 and the mu-head slots 2-3
  - tanh/exp for group g-1 (emitted mid-unit, freeing the z PSUM early;
    the mu PSUM is freed by q(g-1) at the start of unit g+1)

Engine split per group (ACT/DVE balanced ~96% of PE pace, Pool ~89%):
  ACT : leaky x ~4.3 (Prelu, bias fused) + tanh + exp (->iv fp16, accum B)
  DVE : leaky x ~3.7 (custom op from PSUM) + q = (mups+b2m)*iv (fused
        AFFINE_MUL_REDUCE, accum C) + 4x-mode tensor_scalar sums of T/P1/P2
  Pool: fp16 products y2=y*y, p1t=iv*y2, p2t=q*y, and the S sum (SBUF-only)

Precision: fp16 tensors everywhere except PSUM, mu (f32 inside the fused
affine), u=tanh (f32), and the f32 partial accumulators; T sums the same
rounded fp16 y^2 tile that P1's product consumes (bias cancellation).
"""

import numpy as np

N_CORES = 8
N = 131072
D = 128
X_DIM = 128
H2 = 512
M = N // N_CORES          # rows per core = 16384
RG = 1024                 # rows per group
NG = M // RG              # groups per core = 16
NEG_SLOPE = 0.2

# Per-group leaky->DVE unit assignment (units indexed u = c*2 + k in emission
# order).  Uniform 4/4 split: a lopsided unit overloads one engine past PE
# pace and the resulting evacuation backlog stalls the PE.
LEAKY_DVE = (1, 3, 5, 6)

# B = sum(iv) accumulated for free on the ACT exp op (sums pre-rounding f32
# exp values; P1/C consume the rounded fp16 iv).  False spends a cheap 4x
# DVE tensor_scalar on an exactly-consistent B instead.
USE_EXP_ACCUM_B = True


def _leaky_on_dve(g, u):
    return u in (LEAKY_DVE_SPARSE if g % 2 == 1 else LEAKY_DVE)


_leaky_op = None


def _get_leaky_op():
    """Custom DVE uop: out = max((in0 + s0) * imm2, in0 + s0) — fused
    bias-add + leaky-relu in one 1x pass straight from PSUM."""
    global _leaky_op
    if _leaky_op is not None:
        return _leaky_op
    import concourse.dve_ops as DO
    from concourse.dve_spec import C0, C2, Spec, Src0, maxx

    op = DO.DveOp(
        "LEAKY_BIAS_ANT",
        Spec(
            body=maxx((Src0 + C0) * C2, Src0 + C0),
            reference=lambda in0, in1, s0, s1, imm2: np.maximum(
                (in0.astype(np.float32) + s0) * imm2,
                in0.astype(np.float32) + s0),
        ),
        subdim=False,
        uops_sha={"v3": "28ce115f5da0f06f", "v4": ""},
    )
    DO.OPS.append(op)
    DO.CUSTOM_DVE_SPECS[op.name] = op.spec
    DO._SUB_OPCODE_FOR_NAME[op.name] = DO._CUSTOM_DVE_ROW_BASE + len(DO.OPS) - 1
    assert DO._SUB_OPCODE_FOR_NAME[op.name] < 0x20
    _leaky_op = op
    return op


_compiled = None


def _build():
    import concourse.bacc as bacc
    import concourse.tile as tile
    import concourse.mybir as mybir

    F32 = mybir.dt.float32
    F16 = mybir.dt.float16
    AF = mybir.ActivationFunctionType
    OP = mybir.AluOpType

    nc = bacc.Bacc("TRN2", target_bir_lowering=False, debug=False,
                   num_devices=N_CORES)

    x_d = nc.dram_tensor("x16", [M, X_DIM], F16, kind="ExternalInput")
    y_d = nc.dram_tensor("y16", [M, D], F16, kind="ExternalInput")
    y2_d = nc.dram_tensor("y2_16", [M, D], F16, kind="ExternalInput")
    # w1pack cols: [0:512) W1m, [512:1024) W1v; w2pack likewise with
    # W2'[p, c*128+d] = W2[c*128+p, d].  Split so L1(0) can start before the
    # W2 payload lands.
    w1pack_d = nc.dram_tensor("w1pack16", [128, 1024], F16, kind="ExternalInput")
    w2pack_d = nc.dram_tensor("w2pack16", [128, 1024], F16, kind="ExternalInput")
    # bpack cols: [0:4) b1m', [4:8) b1v' (b1'[p,c] = b1[c*128+p]), [8] b2m,
    # [9] -b2v
    bpack_d = nc.dram_tensor("bpack32", [128, 10], F32, kind="ExternalInput")
    out_d = nc.dram_tensor("out", [6, D, NG], F32, kind="ExternalOutput")

    leaky_op = _get_leaky_op()

    with tile.TileContext(nc) as tc:
        with (
            tc.tile_pool(name="consts", bufs=1) as consts,
            tc.tile_pool(name="xtp", bufs=2) as xtp,
            tc.tile_pool(name="ytp", bufs=4) as ytp,
            tc.tile_pool(name="hidden", bufs=3) as hidden,
            tc.tile_pool(name="l2", bufs=4) as l2pool,
            tc.tile_pool(name="junk", bufs=2) as junk,
            tc.tile_pool(name="hpsum", bufs=2, space="PSUM") as hpsum,
            tc.tile_pool(name="l2psum", bufs=1, space="PSUM") as l2psum,
        ):
            # --- startup: packed const DMAs interleaved with first loads ---
            w1p = consts.tile([128, 1024], F16, tag="w1p")
            bp = consts.tile([128, 10], F32, tag="bp")
            w2p = consts.tile([128, 1024], F16, tag="w2p")

            def w1(k, c):
                return w1p[:, k * 512 + c * 128:k * 512 + (c + 1) * 128]

            def w2(k, c):
                return w2p[:, k * 512 + c * 128:k * 512 + (c + 1) * 128]

            def b1(k, c):
                return bp[:, k * 4 + c:k * 4 + c + 1]

            b2m = bp[:, 8:9]
            nb2v = bp[:, 9:10]

            def load_group(g):
                xT = xtp.tile([X_DIM, RG], F16, tag="xT")
                yT = ytp.tile([D, RG], F16, tag="yT")
                y2T = ytp.tile([D, RG], F16, tag="y2T")
                rows = slice(g * RG, (g + 1) * RG)
                nc.sync.dma_start_transpose(xT[:], x_d[rows, :])
                nc.sync.dma_start_transpose(yT[:], y_d[rows, :])
                nc.sync.dma_start_transpose(y2T[:], y2_d[rows, :])
                return xT, yT, y2T

            acc = {}
            for nm in ("P1", "P2", "C", "B", "S", "T"):
                acc[nm] = consts.tile([D, NG], F32, tag=f"acc_{nm}",
                                      name=f"acc_{nm}")
            # Prime the ACT function table (Prelu/Tanh/Exp set) during the
            # startup DMA shadow: the 1.3us LoadActFuncSet otherwise lands in
            # front of the first leaky and stalls the PE.
            warm = consts.tile([128, 1], F32, tag="warm")
            nc.vector.memset(warm[:], 1.0)
            nc.scalar.activation(warm[:], warm[:], AF.Exp)

            loads = [load_group(0)]
            nc.sync.dma_start(w1p[:], w1pack_d[:])
            nc.sync.dma_start(bp[:], bpack_d[:])
            nc.sync.dma_start(w2p[:], w2pack_d[:])
            loads.append(load_group(1))
            hts_hist = {}     # g -> dict u -> ht tile
            iv_hist = {}      # g -> iv tile
            q_hist = {}       # g -> q tile
            mups_hist = {}    # g -> mups psum tile

            def emit_L1_chunk(g, c, xT):
                for k in range(2):
                    # The zps PSUM tile sits idle from tanh(g-1) (mid-unit)
                    # until L2(g)-z (next unit): let the last L1 chunk borrow
                    # it so the hp ring is effectively 3 deep.
                    hp = hpsum.tile([128, RG], F32, tag="hp")
                    for s in range(2):
                        nc.tensor.matmul(hp[:, s * 512:(s + 1) * 512],
                                         w1(k, c),
                                         xT[:, s * 512:(s + 1) * 512],
                                         start=True, stop=True)
                    ht = hidden.tile([128, RG], F16, tag=f"hT{k}{c}")
                    if _leaky_on_dve(g, c * 2 + k):
                        nc.vector._custom_dve(
                            leaky_op, out=ht[:], in0=hp[:],
                            s0=b1(k, c), imm2=NEG_SLOPE)
                    else:
                        nc.scalar.activation(ht[:], hp[:], AF.Prelu,
                                             bias=b1(k, c),
                                             scale=1.0, alpha=NEG_SLOPE)
                    hts_hist[g][c * 2 + k] = ht

            def emit_L2_slot(slot, hts, mups, zps):
                # slots 0,1 -> z-head (k=1), slots 2,3 -> mu-head (k=0)
                k = 1 if slot < 2 else 0
                ps = zps if k == 1 else mups
                for c in ((0, 1) if slot % 2 == 0 else (2, 3)):
                    for s in range(2):
                        nc.tensor.matmul(ps[:, s * 512:(s + 1) * 512],
                                         w2(k, c),
                                         hts[c * 2 + k][:, s * 512:(s + 1) * 512],
                                         start=(c == 0), stop=(c == 3))

            u_hist = {}

            def emit_tanh(g, zps):
                u = l2pool.tile([D, RG], F32, tag="u")
                nc.scalar.activation(u[:], zps[:], AF.Tanh,
                                     bias=nb2v, scale=-1.0)
                u_hist[g] = u

            def emit_exp(g):
                # iv stays f32: the fp16 rounding of iv was the dominant
                # error term (2.3e-2 vs 2.9e-3 measured in emulation).  The
                # Pool tensor_tensor cost is dtype-blind so p1t is no more
                # expensive, and B/C consistency is automatic.
                iv = l2pool.tile([D, RG], F32, tag="iv")
                if USE_EXP_ACCUM_B:
                    nc.scalar.activation(iv[:], u_hist.pop(g)[:], AF.Exp,
                                         accum_out=acc["B"][:, g:g + 1])
                else:
                    nc.scalar.activation(iv[:], u_hist.pop(g)[:], AF.Exp)
                iv_hist[g] = iv

            def emit_products_head(g):
                """Start-of-unit portion for group g (inputs one unit old):
                q frees the mu PSUM and accumulates C; Pool computes the
                p1/p2 products via tensor_tensor (the only legal Pool
                elementwise op); S rides a SWDGE accumulate-DMA."""
                iv, yT, y2T = iv_hist[g], loads[g][1], loads[g][2]
                mups = mups_hist.pop(g)
                q = l2pool.tile([D, RG], F16, tag="q")
                nc.vector.affine_mul_reduce(
                    out=q[:], accum_out=acc["C"][:, g:g + 1],
                    in0=mups[:], in1=iv[:], scale=1.0, bias=b2m)
                q_hist[g] = q
                jS = junk.tile([D, RG], F16, tag="jS")
                nc.vector.tensor_scalar(
                    out=jS[:], in0=yT[:], scalar1=1.0, scalar2=None,
                    op0=OP.mult, op1=OP.add,
                    accum_out=acc["S"][:, g:g + 1])
                p1t = l2pool.tile([D, RG], F16, tag="p1t")
                nc.gpsimd.tensor_tensor(out=p1t[:], in0=iv[:], in1=y2T[:],
                                        op=OP.mult)
                p2t = l2pool.tile([D, RG], F16, tag="p2t")
                nc.gpsimd.tensor_tensor(out=p2t[:], in0=q[:], in1=yT[:],
                                        op=OP.mult)
                return p1t, p2t

            def emit_sums_tail(g, p1t, p2t):
                """End-of-unit 4x DVE sums (T from the preloaded y2T)."""
                for src, nm in ((loads[g][2], "T"), (p1t, "P1"), (p2t, "P2")):
                    j = junk.tile([D, RG], F16, tag=f"j{nm}")
                    nc.vector.tensor_scalar(
                        out=j[:], in0=src[:], scalar1=1.0, scalar2=None,
                        op0=OP.mult, op1=OP.add,
                        accum_out=acc[nm][:, g:g + 1])

            prods = None
            for g in range(NG):
                if g + 2 < NG:
                    loads.append(load_group(g + 2))
                if g >= 2:
                    prods = emit_products_head(g - 2)
                hts_hist[g] = {}
                if g >= 1:
                    mups = l2psum.tile([D, RG], F32, tag="mups")
                    zps = l2psum.tile([D, RG], F32, tag="zps")
                    mups_hist[g - 1] = mups
                if g >= 1:
                    emit_L1_chunk(g, 0, loads[g][0])
                    emit_L1_chunk(g, 1, loads[g][0])
                    emit_L2_slot(0, hts_hist[g - 1], mups, zps)
                    emit_L1_chunk(g, 2, loads[g][0])
                    emit_L2_slot(1, hts_hist[g - 1], mups, zps)
                    emit_tanh(g - 1, zps)
                    emit_L1_chunk(g, 3, loads[g][0])
                    emit_L2_slot(2, hts_hist[g - 1], mups, zps)
                    emit_L2_slot(3, hts_hist[g - 1], mups, zps)
                    emit_exp(g - 1)
                else:
                    for c in range(4):
                        emit_L1_chunk(g, c, loads[g][0])
                if g >= 2:
                    emit_sums_tail(g - 2, *prods)
                    del hts_hist[g - 2]

            # drain unit NG: L2(NG-1) + tanh/exp(NG-1) + products(NG-2)
            prods = emit_products_head(NG - 2)
            mups = l2psum.tile([D, RG], F32, tag="mups")
            zps = l2psum.tile([D, RG], F32, tag="zps")
            mups_hist[NG - 1] = mups
            for slot in range(4):
                emit_L2_slot(slot, hts_hist[NG - 1], mups, zps)
                if slot == 1:
                    emit_tanh(NG - 1, zps)
            emit_exp(NG - 1)
            emit_sums_tail(NG - 2, *prods)

            # drain unit NG+1: products(NG-1) all on DVE (no Pool round
            # trip), S finalized from the running partial, outputs streamed
            # per-acc as soon as each is final
            gl = NG - 1
            iv, yT, y2T = iv_hist[gl], loads[gl][1], loads[gl][2]
            mups = mups_hist.pop(gl)
            q = l2pool.tile([D, RG], F16, tag="q")
            nc.vector.affine_mul_reduce(
                out=q[:], accum_out=acc["C"][:, gl:gl + 1],
                in0=mups[:], in1=iv[:], scale=1.0, bias=b2m)
            nc.sync.dma_start(out_d[0], acc["C"][:])
            nc.sync.dma_start(out_d[1], acc["B"][:])
            jS = junk.tile([D, RG], F16, tag="jS")
            nc.vector.tensor_scalar(
                out=jS[:], in0=yT[:], scalar1=1.0, scalar2=None,
                op0=OP.mult, op1=OP.add,
                accum_out=acc["S"][:, gl:gl + 1])
            nc.sync.dma_start(out_d[2], acc["S"][:])
            jT = junk.tile([D, RG], F16, tag="jT")
            nc.vector.tensor_scalar(
                out=jT[:], in0=y2T[:], scalar1=1.0, scalar2=None,
                op0=OP.mult, op1=OP.add,
                accum_out=acc["T"][:, gl:gl + 1])
            nc.sync.dma_start(out_d[3], acc["T"][:])
            p1t = l2pool.tile([D, RG], F16, tag="p1t")
            nc.vector.tensor_tensor(out=p1t[:], in0=iv[:], in1=y2T[:],
                                    op=OP.mult)
            jP1 = junk.tile([D, RG], F16, tag="jP1")
            nc.vector.tensor_scalar(
                out=jP1[:], in0=p1t[:], scalar1=1.0, scalar2=None,
                op0=OP.mult, op1=OP.add,
                accum_out=acc["P1"][:, gl:gl + 1])
            nc.sync.dma_start(out_d[4], acc["P1"][:])
            p2t = l2pool.tile([D, RG], F16, tag="p2t")
            nc.vector.tensor_tensor(out=p2t[:], in0=q[:], in1=yT[:],
                                    op=OP.mult)
            jP2 = junk.tile([D, RG], F16, tag="jP2")
            nc.vector.tensor_scalar(
                out=jP2[:], in0=p2t[:], scalar1=1.0, scalar2=None,
                op0=OP.mult, op1=OP.add,
                accum_out=acc["P2"][:, gl:gl + 1])
            nc.sync.dma_start(out_d[5], acc["P2"][:])

    nc.compile()
    return nc


def _get_compiled():
    global _compiled
    if _compiled is None:
        _compiled = _build()
    return _compiled


def make_in_maps(x_samples, y_samples, W1m, b1m, W2m, b2m, W1v, b1v, W2v, b2v):
    """Host-side staging: shard x/y over cores, cast to fp16, pack weights."""
    f16 = np.float16
    f32 = np.float32

    def w2_shuffle(W2):
        return (np.asarray(W2, f32).reshape(4, 128, D).transpose(1, 0, 2)
                .reshape(128, 4 * D))

    w1pack = np.concatenate([
        np.asarray(W1m, f32), np.asarray(W1v, f32)], axis=1).astype(f16)
    w2pack = np.concatenate([
        w2_shuffle(W2m), w2_shuffle(W2v)], axis=1).astype(f16)
    bpack = np.concatenate([
        np.asarray(b1m, f32).reshape(4, 128).T,
        np.asarray(b1v, f32).reshape(4, 128).T,
        np.asarray(b2m, f32).reshape(128, 1),
        -np.asarray(b2v, f32).reshape(128, 1)], axis=1)
    shared = {
        "w1pack16": np.ascontiguousarray(w1pack),
        "w2pack16": np.ascontiguousarray(w2pack),
        "bpack32": np.ascontiguousarray(bpack.astype(f32)),
    }
    xs = np.asarray(x_samples, f32).astype(f16)
    ys = np.asarray(y_samples, f32).astype(f16)
    # y^2 rounded exactly as a device op would: f32 square of the f16
    # values, rounded to f16.  T and P1 both consume these same values.
    y2s = (ys.astype(f32) ** 2).astype(f16)
    in_maps = []
    for i in range(N_CORES):
        sl = slice(i * M, (i + 1) * M)
        m = {"x16": np.ascontiguousarray(xs[sl]),
             "y16": np.ascontiguousarray(ys[sl]),
             "y2_16": np.ascontiguousarray(y2s[sl])}
        m.update(shared)
        in_maps.append(m)
    return in_maps


def kernel(x_samples, y_samples, W1m, b1m, W2m, b2m, W1v, b1v, W2v, b2v):
    from concourse.bass_utils import run_bass_kernel_spmd

    nc = _get_compiled()
    in_maps = make_in_maps(x_samples, y_samples, W1m, b1m, W2m, b2m,
                           W1v, b1v, W2v, b2v)
    res = run_bass_kernel_spmd(nc, in_maps, list(range(N_CORES)))
    return combine([r["out"] for r in res.results])


def combine(outs):
    """Host-side gather: sum per-core [6, 128, NG] partials and finish the loss."""
    tot = np.sum([o.astype(np.float64) for o in outs], axis=(0, 3))
    C, B, S, T, P1, P2 = tot
    ym = S / N
    y2m = T / N
    total = P1.sum() - 2.0 * P2.sum() - (y2m * B).sum() + 2.0 * (ym * C).sum()
    return np.float32(-0.5 * total / N)


# revision 34
# speedup vs baseline: 1.0369x; 1.0369x over previous
"""Trainium2 Bass kernel for nn_CLUB_816043786555 (CLUB loss).

Full-input contract: kernel(**inputs) takes the complete arrays, shards the
batch dim across 8 NeuronCores, runs a Bass/Tile kernel per core, and
combines tiny per-core partial sums on the host.

Math: with mu = leaky(x@W1m+b1m)@W2m+b2m, logvar = tanh(leaky(x@W1v+b1v)@W2v+b2v),
iv = exp(-logvar), ym_d = mean_i y, y2m_d = mean_i y^2:

  loss = -0.5/N * sum_{i,d} iv*(y^2 - 2*mu*y - y2m + 2*mu*ym)
       = -0.5/N * [ P1 - 2*P2 - sum_d y2m_d*B_d + 2*sum_d ym_d*C_d ]

with per-core partials P1 = sum iv*y^2, P2 = sum iv*mu*y, C_d = sum_i iv*mu,
B_d = sum_i iv, S_d = sum_i y, T_d = sum_i y^2.  All partials are produced
on-device as fp32 accumulations; the host combine is O(128) work.

Host-side prep (dtype/layout staging only): x/y/W cast to fp16 (identical
rounding to the previous on-device cast path, no DRAM bounce), weights packed
into a single [128,2048] fp16 tensor and biases into [128,10] f32 so startup
is 2 DMAs, b2v negated for tanh's bias slot.

Schedule: groups of RG=1024 rows flow through a 3-deep software pipeline.
Unit g emits:
  - transpose-loads for group g+1
  - product stage for group g-2 (all inputs are then a full unit old, so no
    engine ever head-blocks its in-order queue on a cross-engine dependency)
  - L1(g) matmuls interleaved per 128-wide hidden chunk with L2(g-1)
    matmuls so the PE never gaps (its p-state ramp resets on any gap);
    L2's z-head occupies interleave slots 0-1 and the mu-head slots 2-3
  - tanh/exp for group g-1 (emitted mid-unit, freeing the z PSUM early;
    the mu PSUM is freed by q(g-1) at the start of unit g+1)

Engine split per group (ACT/DVE balanced ~96% of PE pace, Pool ~89%):
  ACT : leaky x ~4.3 (Prelu, bias fused) + tanh + exp (->iv fp16, accum B)
  DVE : leaky x ~3.7 (custom op from PSUM) + q = (mups+b2m)*iv (fused
        AFFINE_MUL_REDUCE, accum C) + 4x-mode tensor_scalar sums of T/P1/P2
  Pool: fp16 products y2=y*y, p1t=iv*y2, p2t=q*y, and the S sum (SBUF-only)

Precision: fp16 tensors everywhere except PSUM, mu (f32 inside the fused
affine), u=tanh (f32), and the f32 partial accumulators; T sums the same
rounded fp16 y^2 tile that P1's product consumes (bias cancellation).
"""

import numpy as np

N_CORES = 8
N = 131072
D = 128
X_DIM = 128
H2 = 512
M = N // N_CORES          # rows per core = 16384
RG = 1024                 # rows per group
NG = M // RG              # groups per core = 16
NEG_SLOPE = 0.2

# Per-group leaky->DVE unit assignment (units indexed u = c*2 + k in emission
# order).  Uniform 4/4 split: a lopsided unit overloads one engine past PE
# pace and the resulting evacuation backlog stalls the PE.
LEAKY_DVE = (1, 3, 5, 6)

# B = sum(iv) accumulated for free on the ACT exp op (sums pre-rounding f32
# exp values; P1/C consume the rounded fp16 iv).  False spends a cheap 4x
# DVE tensor_scalar on an exactly-consistent B instead.
USE_EXP_ACCUM_B = True


def _leaky_on_dve(g, u):
    return u in LEAKY_DVE


_leaky_op = None


def _get_leaky_op():
    """Custom DVE uop: out = max((in0 + s0) * imm2, in0 + s0) — fused
    bias-add + leaky-relu in one 1x pass straight from PSUM."""
    global _leaky_op
    if _leaky_op is not None:
        return _leaky_op
    import concourse.dve_ops as DO
    from concourse.dve_spec import C0, C2, Spec, Src0, maxx

    op = DO.DveOp(
        "LEAKY_BIAS_ANT",
        Spec(
            body=maxx((Src0 + C0) * C2, Src0 + C0),
            reference=lambda in0, in1, s0, s1, imm2: np.maximum(
                (in0.astype(np.float32) + s0) * imm2,
                in0.astype(np.float32) + s0),
        ),
        subdim=False,
        uops_sha={"v3": "28ce115f5da0f06f", "v4": ""},
    )
    DO.OPS.append(op)
    DO.CUSTOM_DVE_SPECS[op.name] = op.spec
    DO._SUB_OPCODE_FOR_NAME[op.name] = DO._CUSTOM_DVE_ROW_BASE + len(DO.OPS) - 1
    assert DO._SUB_OPCODE_FOR_NAME[op.name] < 0x20
    _leaky_op = op
    return op


_compiled = None


def _build():
    import concourse.bacc as bacc
    import concourse.tile as tile
    import concourse.mybir as mybir

    F32 = mybir.dt.float32
    F16 = mybir.dt.float16
    AF = mybir.ActivationFunctionType
    OP = mybir.AluOpType

    nc = bacc.Bacc("TRN2", target_bir_lowering=False, debug=False,
                   num_devices=N_CORES)

    x_d = nc.dram_tensor("x16", [M, X_DIM], F16, kind="ExternalInput")
    y_d = nc.dram_tensor("y16", [M, D], F16, kind="ExternalInput")
    y2_d = nc.dram_tensor("y2_16", [M, D], F16, kind="ExternalInput")
    # w1pack cols: [0:512) W1m, [512:1024) W1v; w2pack likewise with
    # W2'[p, c*128+d] = W2[c*128+p, d].  Split so L1(0) can start before the
    # W2 payload lands.
    w1pack_d = nc.dram_tensor("w1pack16", [128, 1024], F16, kind="ExternalInput")
    w2pack_d = nc.dram_tensor("w2pack16", [128, 1024], F16, kind="ExternalInput")
    # bpack cols: [0:4) b1m', [4:8) b1v' (b1'[p,c] = b1[c*128+p]), [8] b2m,
    # [9] -b2v
    bpack_d = nc.dram_tensor("bpack32", [128, 10], F32, kind="ExternalInput")
    out_d = nc.dram_tensor("out", [6, D, NG], F32, kind="ExternalOutput")

    leaky_op = _get_leaky_op()

    with tile.TileContext(nc) as tc:
        with (
            tc.tile_pool(name="consts", bufs=1) as consts,
            tc.tile_pool(name="xtp", bufs=2) as xtp,
            tc.tile_pool(name="ytp", bufs=4) as ytp,
            tc.tile_pool(name="hidden", bufs=3) as hidden,
            tc.tile_pool(name="l2", bufs=4) as l2pool,
            tc.tile_pool(name="junk", bufs=2) as junk,
            tc.tile_pool(name="hpsum", bufs=2, space="PSUM") as hpsum,
            tc.tile_pool(name="l2psum", bufs=1, space="PSUM") as l2psum,
        ):
            # --- startup: packed const DMAs interleaved with first loads ---
            w1p = consts.tile([128, 1024], F16, tag="w1p")
            bp = consts.tile([128, 10], F32, tag="bp")
            w2p = consts.tile([128, 1024], F16, tag="w2p")

            def w1(k, c):
                return w1p[:, k * 512 + c * 128:k * 512 + (c + 1) * 128]

            def w2(k, c):
                return w2p[:, k * 512 + c * 128:k * 512 + (c + 1) * 128]

            def b1(k, c):
                return bp[:, k * 4 + c:k * 4 + c + 1]

            b2m = bp[:, 8:9]
            nb2v = bp[:, 9:10]

            def load_group(g):
                xT = xtp.tile([X_DIM, RG], F16, tag="xT")
                yT = ytp.tile([D, RG], F16, tag="yT")
                y2T = ytp.tile([D, RG], F16, tag="y2T")
                rows = slice(g * RG, (g + 1) * RG)
                nc.sync.dma_start_transpose(xT[:], x_d[rows, :])
                nc.sync.dma_start_transpose(yT[:], y_d[rows, :])
                nc.sync.dma_start_transpose(y2T[:], y2_d[rows, :])
                return xT, yT, y2T

            acc = {}
            for nm in ("P1", "P2", "C", "B", "S", "T"):
                acc[nm] = consts.tile([D, NG], F32, tag=f"acc_{nm}",
                                      name=f"acc_{nm}")
            # Prime the ACT function table (Prelu/Tanh/Exp set) during the
            # startup DMA shadow: the 1.3us LoadActFuncSet otherwise lands in
            # front of the first leaky and stalls the PE.
            warm = consts.tile([128, 1], F32, tag="warm")
            nc.vector.memset(warm[:], 1.0)
            nc.scalar.activation(warm[:], warm[:], AF.Exp)

            loads = [load_group(0)]
            nc.sync.dma_start(w1p[:], w1pack_d[:])
            nc.sync.dma_start(bp[:], bpack_d[:])
            nc.sync.dma_start(w2p[:], w2pack_d[:])
            loads.append(load_group(1))
            hts_hist = {}     # g -> dict u -> ht tile
            iv_hist = {}      # g -> iv tile
            mups_hist = {}    # g -> mups psum tile

            def emit_L1_chunk(g, c, xT):
                for k in range(2):
                    # The zps PSUM tile sits idle from tanh(g-1) (mid-unit)
                    # until L2(g)-z (next unit): let the last L1 chunk borrow
                    # it so the hp ring is effectively 3 deep.
                    hp = hpsum.tile([128, RG], F32, tag="hp")
                    for s in range(2):
                        nc.tensor.matmul(hp[:, s * 512:(s + 1) * 512],
                                         w1(k, c),
                                         xT[:, s * 512:(s + 1) * 512],
                                         start=True, stop=True)
                    ht = hidden.tile([128, RG], F16, tag=f"hT{k}{c}")
                    if _leaky_on_dve(g, c * 2 + k):
                        nc.vector._custom_dve(
                            leaky_op, out=ht[:], in0=hp[:],
                            s0=b1(k, c), imm2=NEG_SLOPE)
                    else:
                        nc.scalar.activation(ht[:], hp[:], AF.Prelu,
                                             bias=b1(k, c),
                                             scale=1.0, alpha=NEG_SLOPE)
                    hts_hist[g][c * 2 + k] = ht

            def emit_L2_slot(slot, hts, mups, zps):
                # slots 0,1 -> z-head (k=1), slots 2,3 -> mu-head (k=0)
                k = 1 if slot < 2 else 0
                ps = zps if k == 1 else mups
                for c in ((0, 1) if slot % 2 == 0 else (2, 3)):
                    for s in range(2):
                        nc.tensor.matmul(ps[:, s * 512:(s + 1) * 512],
                                         w2(k, c),
                                         hts[c * 2 + k][:, s * 512:(s + 1) * 512],
                                         start=(c == 0), stop=(c == 3))

            u_hist = {}

            def emit_tanh(g, zps):
                u = l2pool.tile([D, RG], F32, tag="u")
                nc.scalar.activation(u[:], zps[:], AF.Tanh,
                                     bias=nb2v, scale=-1.0)
                u_hist[g] = u

            def emit_exp(g):
                # iv stays f32: the fp16 rounding of iv was the dominant
                # error term (2.3e-2 vs 2.9e-3 measured in emulation).  The
                # Pool tensor_tensor cost is dtype-blind so p1t is no more
                # expensive, and B/C consistency is automatic.
                iv = l2pool.tile([D, RG], F32, tag="iv")
                if USE_EXP_ACCUM_B:
                    nc.scalar.activation(iv[:], u_hist.pop(g)[:], AF.Exp,
                                         accum_out=acc["B"][:, g:g + 1])
                else:
                    nc.scalar.activation(iv[:], u_hist.pop(g)[:], AF.Exp)
                iv_hist[g] = iv

            def emit_products_head(g):
                """Start-of-unit portion for group g (inputs one unit old):
                q frees the mu PSUM and accumulates C; Pool computes the
                p1/p2 products via tensor_tensor (the only legal Pool
                elementwise op); S rides a SWDGE accumulate-DMA."""
                iv, yT, y2T = iv_hist[g], loads[g][1], loads[g][2]
                mups = mups_hist.pop(g)
                q = l2pool.tile([D, RG], F16, tag="q")
                nc.vector.affine_mul_reduce(
                    out=q[:], accum_out=acc["C"][:, g:g + 1],
                    in0=mups[:], in1=iv[:], scale=1.0, bias=b2m)
                jS = junk.tile([D, RG], F16, tag="jS")
                nc.vector.tensor_scalar(
                    out=jS[:], in0=yT[:], scalar1=1.0, scalar2=None,
                    op0=OP.mult, op1=OP.add,
                    accum_out=acc["S"][:, g:g + 1])
                p1t = l2pool.tile([D, RG], F16, tag="p1t")
                nc.gpsimd.tensor_tensor(out=p1t[:], in0=iv[:], in1=y2T[:],
                                        op=OP.mult)
                p2t = l2pool.tile([D, RG], F16, tag="p2t")
                nc.gpsimd.tensor_tensor(out=p2t[:], in0=q[:], in1=yT[:],
                                        op=OP.mult)
                return p1t, p2t

            def emit_sums_tail(g, p1t, p2t):
                """End-of-unit 4x DVE sums (T from the preloaded y2T)."""
                for src, nm in ((loads[g][2], "T"), (p1t, "P1"), (p2t, "P2")):
                    j = junk.tile([D, RG], F16, tag=f"j{nm}")
                    nc.vector.tensor_scalar(
                        out=j[:], in0=src[:], scalar1=1.0, scalar2=None,
                        op0=OP.mult, op1=OP.add,
                        accum_out=acc[nm][:, g:g + 1])

            prods = None
            for g in range(NG):
                if g + 2 < NG:
                    loads.append(load_group(g + 2))
                if g >= 2:
                    prods = emit_products_head(g - 2)
                hts_hist[g] = {}
                if g >= 1:
                    mups = l2psum.tile([D, RG], F32, tag="mups")
                    zps = l2psum.tile([D, RG], F32, tag="zps")
                    mups_hist[g - 1] = mups
                if g >= 1:
                    emit_L1_chunk(g, 0, loads[g][0])
                    emit_L1_chunk(g, 1, loads[g][0])
                    emit_L2_slot(0, hts_hist[g - 1], mups, zps)
                    emit_L1_chunk(g, 2, loads[g][0])
                    emit_L2_slot(1, hts_hist[g - 1], mups, zps)
                    emit_tanh(g - 1, zps)
                    emit_L1_chunk(g, 3, loads[g][0])
                    emit_L2_slot(2, hts_hist[g - 1], mups, zps)
                    emit_L2_slot(3, hts_hist[g - 1], mups, zps)
                    emit_exp(g - 1)
                else:
                    for c in range(4):
                        emit_L1_chunk(g, c, loads[g][0])
                if g >= 2:
                    emit_sums_tail(g - 2, *prods)
                    del hts_hist[g - 2]

            # drain unit NG: L2(NG-1) + tanh/exp(NG-1) + products(NG-2)
            prods = emit_products_head(NG - 2)
            mups = l2psum.tile([D, RG], F32, tag="mups")
            zps = l2psum.tile([D, RG], F32, tag="zps")
            mups_hist[NG - 1] = mups
            for slot in range(4):
                emit_L2_slot(slot, hts_hist[NG - 1], mups, zps)
                if slot == 1:
                    emit_tanh(NG - 1, zps)
            emit_exp(NG - 1)
            emit_sums_tail(NG - 2, *prods)

            # drain unit NG+1: products(NG-1) all on DVE (no Pool round
            # trip), S finalized from the running partial, outputs streamed
            # per-acc as soon as each is final
            gl = NG - 1
            iv, yT, y2T = iv_hist[gl], loads[gl][1], loads[gl][2]
            mups = mups_hist.pop(gl)
            q = l2pool.tile([D, RG], F16, tag="q")
            nc.vector.affine_mul_reduce(
                out=q[:], accum_out=acc["C"][:, gl:gl + 1],
                in0=mups[:], in1=iv[:], scale=1.0, bias=b2m)
            nc.sync.dma_start(out_d[0], acc["C"][:])
            nc.sync.dma_start(out_d[1], acc["B"][:])
            jS = junk.tile([D, RG], F16, tag="jS")
            nc.vector.tensor_scalar(
                out=jS[:], in0=yT[:], scalar1=1.0, scalar2=None,
                op0=OP.mult, op1=OP.add,
                accum_out=acc["S"][:, gl:gl + 1])
            nc.sync.dma_start(out_d[2], acc["S"][:])
            jT = junk.tile([D, RG], F16, tag="jT")
            nc.vector.tensor_scalar(
                out=jT[:], in0=y2T[:], scalar1=1.0, scalar2=None,
                op0=OP.mult, op1=OP.add,
                accum_out=acc["T"][:, gl:gl + 1])
            nc.sync.dma_start(out_d[3], acc["T"][:])
            p1t = l2pool.tile([D, RG], F16, tag="p1t")
            nc.vector.tensor_tensor(out=p1t[:], in0=iv[:], in1=y2T[:],
                                    op=OP.mult)
            jP1 = junk.tile([D, RG], F16, tag="jP1")
            nc.vector.tensor_scalar(
                out=jP1[:], in0=p1t[:], scalar1=1.0, scalar2=None,
                op0=OP.mult, op1=OP.add,
                accum_out=acc["P1"][:, gl:gl + 1])
            nc.sync.dma_start(out_d[4], acc["P1"][:])
            p2t = l2pool.tile([D, RG], F16, tag="p2t")
            nc.vector.tensor_tensor(out=p2t[:], in0=q[:], in1=yT[:],
                                    op=OP.mult)
            jP2 = junk.tile([D, RG], F16, tag="jP2")
            nc.vector.tensor_scalar(
                out=jP2[:], in0=p2t[:], scalar1=1.0, scalar2=None,
                op0=OP.mult, op1=OP.add,
                accum_out=acc["P2"][:, gl:gl + 1])
            nc.sync.dma_start(out_d[5], acc["P2"][:])

    nc.compile()
    return nc


def _get_compiled():
    global _compiled
    if _compiled is None:
        _compiled = _build()
    return _compiled


def make_in_maps(x_samples, y_samples, W1m, b1m, W2m, b2m, W1v, b1v, W2v, b2v):
    """Host-side staging: shard x/y over cores, cast to fp16, pack weights."""
    f16 = np.float16
    f32 = np.float32

    def w2_shuffle(W2):
        return (np.asarray(W2, f32).reshape(4, 128, D).transpose(1, 0, 2)
                .reshape(128, 4 * D))

    w1pack = np.concatenate([
        np.asarray(W1m, f32), np.asarray(W1v, f32)], axis=1).astype(f16)
    w2pack = np.concatenate([
        w2_shuffle(W2m), w2_shuffle(W2v)], axis=1).astype(f16)
    bpack = np.concatenate([
        np.asarray(b1m, f32).reshape(4, 128).T,
        np.asarray(b1v, f32).reshape(4, 128).T,
        np.asarray(b2m, f32).reshape(128, 1),
        -np.asarray(b2v, f32).reshape(128, 1)], axis=1)
    shared = {
        "w1pack16": np.ascontiguousarray(w1pack),
        "w2pack16": np.ascontiguousarray(w2pack),
        "bpack32": np.ascontiguousarray(bpack.astype(f32)),
    }
    xs = np.asarray(x_samples, f32).astype(f16)
    ys = np.asarray(y_samples, f32).astype(f16)
    # y^2 rounded exactly as a device op would: f32 square of the f16
    # values, rounded to f16.  T and P1 both consume these same values.
    y2s = (ys.astype(f32) ** 2).astype(f16)
    in_maps = []
    for i in range(N_CORES):
        sl = slice(i * M, (i + 1) * M)
        m = {"x16": np.ascontiguousarray(xs[sl]),
             "y16": np.ascontiguousarray(ys[sl]),
             "y2_16": np.ascontiguousarray(y2s[sl])}
        m.update(shared)
        in_maps.append(m)
    return in_maps


def kernel(x_samples, y_samples, W1m, b1m, W2m, b2m, W1v, b1v, W2v, b2v):
    from concourse.bass_utils import run_bass_kernel_spmd

    nc = _get_compiled()
    in_maps = make_in_maps(x_samples, y_samples, W1m, b1m, W2m, b2m,
                           W1v, b1v, W2v, b2v)
    res = run_bass_kernel_spmd(nc, in_maps, list(range(N_CORES)))
    return combine([r["out"] for r in res.results])


def combine(outs):
    """Host-side gather: sum per-core [6, 128, NG] partials and finish the loss."""
    tot = np.sum([o.astype(np.float64) for o in outs], axis=(0, 3))
    C, B, S, T, P1, P2 = tot
    ym = S / N
    y2m = T / N
    total = P1.sum() - 2.0 * P2.sum() - (y2m * B).sum() + 2.0 * (ym * C).sum()
    return np.float32(-0.5 * total / N)
